# revision 10
# baseline (speedup 1.0000x reference)
"""Multi-head attention (B=2, T=2048, D=1024, H=16) on 8 TRN2 NeuronCores.

Sharding: core c handles batch b = c//4 and heads 4*(c%4) .. 4*(c%4)+3.
Each core computes Q/K/V projections for its 4 heads, attention scores in
transposed layout (scoresT[k, q]), softmax via exp + a ones-column appended to
V (so the denominator falls out of the attn@V matmul), the attention-weighted
values, and a partial output projection. The host assembles the attention
weights (transpose of the device layout) and sums the 4 per-core W_o partials
per batch.

Matmuls run in fp16 (10-bit mantissa, full PE rate); the projections
accumulate in fp32 PSUM; exp on ScalarE; normalization in fp32.
"""
import numpy as np

import concourse.bacc as bacc
import concourse.mybir as mybir
from concourse import tile
from concourse.bass_utils import run_bass_kernel_spmd

T = 2048
D = 1024
HL = 4  # heads per core
NC = 8  # D / 128 contraction chunks
NT = 16  # T / 128 tiles (t-tiles and k-tiles)
NQB = 4  # T / 512 query blocks
F32 = mybir.dt.float32
MMDT = mybir.dt.float16  # matmul operand dtype
NPDT = np.float16
EXP = mybir.ActivationFunctionType.Exp
SCALE = 0.125  # 1/sqrt(d_k)

LAST_RESULT = None  # stashed BassKernelResults for test harnesses


def _build():
    nc = bacc.Bacc("TRN2", target_bir_lowering=False, debug=False, num_devices=8)

    xT = nc.dram_tensor("xT", [D, T], MMDT, kind="ExternalInput").ap()
    wq = nc.dram_tensor("wq", [D, 256], MMDT, kind="ExternalInput").ap()
    wk = nc.dram_tensor("wk", [D, 256], MMDT, kind="ExternalInput").ap()
    wv = nc.dram_tensor("wv", [D, 256], MMDT, kind="ExternalInput").ap()
    wo = nc.dram_tensor("wo", [256, D], MMDT, kind="ExternalInput").ap()
    onesd = nc.dram_tensor("ones", [128, 64], MMDT, kind="ExternalInput").ap()
    attnT = nc.dram_tensor("attnT", [HL, T, T], F32, kind="ExternalOutput").ap()
    outp = nc.dram_tensor("outp", [T, D], F32, kind="ExternalOutput").ap()

    with tile.TileContext(nc) as tc:
        with tc.tile_pool(name="persist", bufs=1) as P:
            qt = [P.tile([128, T], MMDT, tag=f"qt{p}", name=f"qt{p}") for p in range(2)]
            kt_ = [P.tile([128, T], MMDT, tag=f"kt{p}", name=f"kt{p}") for p in range(2)]
            v4 = P.tile([128, NT * 260], MMDT, tag="v4")  # [V_l | 1] per (tt, l)
            ot2 = [P.tile([128, T], MMDT, tag=f"ot{p}", name=f"ot{p}") for p in range(2)]
            wot = P.tile([128, 2 * D], MMDT, tag="wot")

            # ones columns of v4: position tt*260 + l*65 + 64
            v4g = v4[:].rearrange("p (g j) -> p g j", j=65)  # g = tt*4 + l
            nc.sync.dma_start(v4g[:, :, 64:65], onesd.unsqueeze(2))
            nc.sync.dma_start(
                wot[:].rearrange("p (c t) -> p c t", t=D),
                wo.rearrange("(c p) t -> p c t", p=128),
            )

            # ---------------- phase 1: load x, project Q/K/V ----------------
            with (
                tc.tile_pool(name="ph1", bufs=1) as L,
                tc.tile_pool(name="ps1", bufs=4, space="PSUM") as PS1,
                tc.tile_pool(name="psv", bufs=2, space="PSUM") as PSV,
            ):
                xtb = L.tile([128, NC * T], MMDT, tag="xtb")
                nc.sync.dma_start(
                    xtb[:].rearrange("p (c t) -> p c t", t=T),
                    xT.rearrange("(c p) t -> p c t", p=128),
                )
                wqt = L.tile([128, NC * 256], MMDT, tag="wqt")
                wkt = L.tile([128, NC * 256], MMDT, tag="wkt")
                wvt = L.tile([128, NC * 256], MMDT, tag="wvt")
                for wtile, wdram in ((wqt, wq), (wkt, wk), (wvt, wv)):
                    nc.sync.dma_start(
                        wtile[:].rearrange("p (c d) -> p c d", d=256),
                        wdram.rearrange("(c p) d -> p c d", p=128),
                    )

                # QT2/KT2: [128 = head-pair d dims, T]; 4 accumulators in flight
                for dst, wtile in ((qt, wqt), (kt_, wkt)):
                    for grp in range(2):
                        outs = [(grp, tb) for tb in range(NQB)]
                        pss = [
                            PS1.tile([128, 512], F32, tag="ps1", name=f"p1_{id(wtile)}_{grp}_{tb}")
                            for tb in range(NQB)
                        ]
                        for c in range(NC):
                            for o, (p, tb) in enumerate(outs):
                                nc.tensor.matmul(
                                    pss[o][:],
                                    wtile[:, c * 256 + p * 128 : c * 256 + p * 128 + 128],
                                    xtb[:, c * T + tb * 512 : c * T + tb * 512 + 512],
                                    start=(c == 0),
                                    stop=(c == NC - 1),
                                )
                        for o, (p, tb) in enumerate(outs):
                            nc.vector.tensor_copy(
                                dst[p][:, tb * 512 : (tb + 1) * 512], pss[o][:]
                            )
                # V: natural layout [t, d], 4 heads side by side (65-strided groups)
                for tg in range(NT // 2):
                    pss = [
                        PSV.tile([128, 256], F32, tag="psv", name=f"pv_{tg}_{o}")
                        for o in range(2)
                    ]
                    for c in range(NC):
                        for o in range(2):
                            tt = 2 * tg + o
                            nc.tensor.matmul(
                                pss[o][:],
                                xtb[:, c * T + tt * 128 : c * T + tt * 128 + 128],
                                wvt[:, c * 256 : (c + 1) * 256],
                                start=(c == 0),
                                stop=(c == NC - 1),
                            )
                    for o in range(2):
                        tt = 2 * tg + o
                        dst = v4[:, tt * 260 : (tt + 1) * 260].rearrange(
                            "p (l j) -> p l j", j=65
                        )[:, :, 0:64]
                        nc.scalar.copy(dst, pss[o][:].rearrange("p (l j) -> p l j", j=64))

            # ---------------- phase 2: attention (+ fused output proj) ----------------
            with (
                tc.tile_pool(name="ph2", bufs=2) as E2,
                tc.tile_pool(name="stg", bufs=1) as STG,
                tc.tile_pool(name="nrm", bufs=2) as NP,
                tc.tile_pool(name="ph3", bufs=3) as S3,
                tc.tile_pool(name="ps2", bufs=1, space="PSUM") as PS2,
                tc.tile_pool(name="pso", bufs=2, space="PSUM") as PSO,
            ):
                for qb in range(NQB):
                    for p in range(2):
                        e01 = [
                            E2.tile([128, NT * 512], MMDT, tag=f"e{l}", name=f"e{l}_{p}_{qb}")
                            for l in range(2)
                        ]
                        po = [
                            PSO.tile([65, 512], F32, tag=f"po{l}", name=f"po{l}_{p}_{qb}")
                            for l in range(2)
                        ]
                        qs = qt[p][:, qb * 512 : (qb + 1) * 512]
                        # (a) all scores + exp, pipelined through 2x2 PSUM banks
                        for ktp in range(8):
                            sAB = [
                                PS2.tile([128, 1024], F32, tag=f"s{l}", name=f"s{l}_{p}_{qb}_{ktp}")
                                for l in range(2)
                            ]
                            for half in range(2):
                                kti = 2 * ktp + half
                                for l in range(2):
                                    lo, hi = 64 * l, 64 * l + 64
                                    nc.tensor.matmul(
                                        sAB[l][:, half * 512 : half * 512 + 512],
                                        kt_[p][lo:hi, kti * 128 : kti * 128 + 128],
                                        qs[lo:hi, :],
                                        start=True,
                                        stop=True,
                                    )
                            for l in range(2):
                                nc.scalar.activation(
                                    e01[l][:, ktp * 1024 : ktp * 1024 + 1024],
                                    sAB[l][:],
                                    EXP,
                                    scale=SCALE,
                                )
                        # (b) attn @ [V|1], dense back-to-back on PE
                        for kti in range(NT):
                            for l in range(2):
                                g0 = kti * 260 + (2 * p + l) * 65
                                nc.tensor.matmul(
                                    po[l][:],
                                    v4[:, g0 : g0 + 65],
                                    e01[l][:, kti * 512 : kti * 512 + 512],
                                    start=(kti == 0),
                                    stop=(kti == NT - 1),
                                    skip_group_check=True,
                                )
                        # (c) normalize: recip of sums (broadcast first), ot2, attn out
                        for l in range(2):
                            e = e01[l]
                            srow = NP.tile([1, 512], F32, tag="srow")
                            nc.vector.tensor_copy(srow[:], po[l][64:65, :])
                            rbraw = NP.tile([128, 1024], F32, tag="rbraw")
                            nc.gpsimd.partition_broadcast(rbraw[:, 0:512], srow[:])
                            nc.gpsimd.partition_broadcast(rbraw[:, 512:1024], srow[:])
                            rb2 = NP.tile([128, 1024], F32, tag="rb2")
                            nc.vector.reciprocal(rb2[:], rbraw[:])
                            # normalized attention-weighted values -> ot2
                            if l == 0:
                                nc.vector.tensor_tensor(
                                    ot2[p][0:64, qb * 512 : qb * 512 + 512],
                                    po[l][0:64, :],
                                    rb2[0:64, 0:512],
                                    op=mybir.AluOpType.mult,
                                )
                            else:
                                tmp = NP.tile([64, 512], MMDT, tag="tmp")
                                nc.vector.tensor_tensor(
                                    tmp[:],
                                    po[l][0:64, :],
                                    rb2[0:64, 0:512],
                                    op=mybir.AluOpType.mult,
                                )
                                nc.sync.dma_start(
                                    ot2[p][64:128, qb * 512 : qb * 512 + 512], tmp[:]
                                )
                            # normalize E into f32 staging halves, write attnT
                            lg = 2 * p + l
                            for hf in range(2):
                                stage = STG.tile(
                                    [128, 8 * 512], F32, tag=f"st{l}{hf}",
                                    name=f"st{l}{hf}_{p}_{qb}",
                                )
                                for kp in range(4):
                                    eng = nc.gpsimd if kp == 3 else nc.vector
                                    eng.tensor_tensor(
                                        stage[:, kp * 1024 : kp * 1024 + 1024],
                                        e[:, hf * 4096 + kp * 1024 : hf * 4096 + kp * 1024 + 1024],
                                        rb2[:],
                                        op=mybir.AluOpType.mult,
                                    )
                                nc.sync.dma_start(
                                    attnT[lg].rearrange("(kt p) q -> p kt q", p=128)[
                                        :, hf * 8 : hf * 8 + 8, qb * 512 : qb * 512 + 512
                                    ],
                                    stage[:].rearrange("p (kt j) -> p kt j", j=512),
                                )
                    # (d) output projection for the 4 t-tiles of this qb
                    for tt in range(4 * qb, 4 * qb + 4):
                        for db in range(2):
                            ps = PS2.tile([128, 512], F32, tag="s0", name=f"p3_{tt}_{db}")
                            for p in range(2):
                                nc.tensor.matmul(
                                    ps[:],
                                    ot2[p][:, tt * 128 : tt * 128 + 128],
                                    wot[:, p * D + db * 512 : p * D + db * 512 + 512],
                                    start=(p == 0),
                                    stop=(p == 1),
                                )
                            st = S3.tile([128, 512], F32, tag="st")
                            nc.vector.tensor_copy(st[:], ps[:])
                            nc.sync.dma_start(
                                outp[tt * 128 : tt * 128 + 128, db * 512 : db * 512 + 512],
                                st[:],
                            )

    nc.compile()
    return nc


def kernel(x, W_q, W_k, W_v, W_o):
    global LAST_RESULT
    x = np.asarray(x, dtype=np.float32)
    W_q = np.asarray(W_q, dtype=np.float32)
    W_k = np.asarray(W_k, dtype=np.float32)
    W_v = np.asarray(W_v, dtype=np.float32)
    W_o = np.asarray(W_o, dtype=np.float32)

    nc = _build()

    ones = np.ones((128, 64), dtype=NPDT)
    in_maps = []
    for c in range(8):
        b = c // 4
        hs = 4 * (c % 4)  # first global head of this core
        r0 = 64 * hs
        in_maps.append(
            {
                "xT": np.ascontiguousarray(x[b].T).astype(NPDT),
                "wq": np.ascontiguousarray(W_q[r0 : r0 + 256, :].T).astype(NPDT),
                "wk": np.ascontiguousarray(W_k[r0 : r0 + 256, :].T).astype(NPDT),
                "wv": np.ascontiguousarray(W_v[r0 : r0 + 256, :].T).astype(NPDT),
                "wo": np.ascontiguousarray(W_o[:, r0 : r0 + 256].T).astype(NPDT),
                "ones": ones,
            }
        )

    res = run_bass_kernel_spmd(nc, in_maps, core_ids=list(range(8)))
    LAST_RESULT = res

    B, H = 2, 16
    out = np.zeros((B, T, D), dtype=np.float32)
    attn = np.empty((B, H, T, T), dtype=np.float32)
    for c in range(8):
        b = c // 4
        hs = 4 * (c % 4)
        r = res.results[c]
        out[b] += r["outp"]
        a = r["attnT"]  # [4, k, q]
        for l in range(HL):
            attn[b, hs + l] = a[l].T
    return out, attn


# revision 12
# speedup vs baseline: 1.0690x; 1.0690x over previous
"""Multi-head attention (B=2, T=2048, D=1024, H=16) on 8 TRN2 NeuronCores.

Sharding: core c handles batch b = c//4 and heads 4*(c%4) .. 4*(c%4)+3.
Each core computes Q/K/V projections for its 4 heads, attention scores in
transposed layout (scoresT[k, q]), softmax via exp + a ones-column appended to
V (so the denominator falls out of the attn@V matmul), the attention-weighted
values, and a partial output projection. The host assembles the attention
weights (transpose of the device layout) and sums the 4 per-core W_o partials
per batch.

Matmuls run in fp16 (10-bit mantissa, full PE rate); the projections
accumulate in fp32 PSUM; exp on ScalarE; normalization in fp32.
"""
import numpy as np

import concourse.bacc as bacc
import concourse.mybir as mybir
from concourse import tile
from concourse.bass_utils import run_bass_kernel_spmd

T = 2048
D = 1024
HL = 4  # heads per core
NC = 8  # D / 128 contraction chunks
NT = 16  # T / 128 tiles (t-tiles and k-tiles)
NQB = 4  # T / 512 query blocks
F32 = mybir.dt.float32
MMDT = mybir.dt.float16  # matmul operand dtype
NPDT = np.float16
EXP = mybir.ActivationFunctionType.Exp
SCALE = 0.125  # 1/sqrt(d_k)

LAST_RESULT = None  # stashed BassKernelResults for test harnesses


def _build():
    nc = bacc.Bacc("TRN2", target_bir_lowering=False, debug=False, num_devices=8)

    xT = nc.dram_tensor("xT", [D, T], MMDT, kind="ExternalInput").ap()
    wq = nc.dram_tensor("wq", [D, 256], MMDT, kind="ExternalInput").ap()
    wk = nc.dram_tensor("wk", [D, 256], MMDT, kind="ExternalInput").ap()
    wv = nc.dram_tensor("wv", [D, 256], MMDT, kind="ExternalInput").ap()
    wo = nc.dram_tensor("wo", [256, D], MMDT, kind="ExternalInput").ap()
    onesd = nc.dram_tensor("ones", [128, 64], MMDT, kind="ExternalInput").ap()
    attnT = nc.dram_tensor("attnT", [HL, T, T], F32, kind="ExternalOutput").ap()
    outp = nc.dram_tensor("outp", [T, D], F32, kind="ExternalOutput").ap()

    with tile.TileContext(nc) as tc:
        with tc.tile_pool(name="persist", bufs=1) as P:
            qt = [P.tile([128, T], MMDT, tag=f"qt{p}", name=f"qt{p}") for p in range(2)]
            kt_ = [P.tile([128, T], MMDT, tag=f"kt{p}", name=f"kt{p}") for p in range(2)]
            v4 = P.tile([128, NT * 260], MMDT, tag="v4")  # [V_l | 1] per (tt, l)
            ot2 = [P.tile([128, T], MMDT, tag=f"ot{p}", name=f"ot{p}") for p in range(2)]
            wot = P.tile([128, 2 * D], MMDT, tag="wot")

            # ones columns of v4: position tt*260 + l*65 + 64
            v4g = v4[:].rearrange("p (g j) -> p g j", j=65)  # g = tt*4 + l
            nc.sync.dma_start(v4g[:, :, 64:65], onesd.unsqueeze(2))
            nc.sync.dma_start(
                wot[:].rearrange("p (c t) -> p c t", t=D),
                wo.rearrange("(c p) t -> p c t", p=128),
            )

            # ---------------- phase 1: load x, project Q/K/V ----------------
            with (
                tc.tile_pool(name="ph1", bufs=1) as L,
                tc.tile_pool(name="ps1", bufs=4, space="PSUM") as PS1,
                tc.tile_pool(name="psv", bufs=2, space="PSUM") as PSV,
            ):
                xtb = L.tile([128, NC * T], MMDT, tag="xtb")
                for c in range(NC):
                    nc.sync.dma_start(
                        xtb[:, c * T : (c + 1) * T], xT[c * 128 : (c + 1) * 128, :]
                    )
                wqt = L.tile([128, NC * 256], MMDT, tag="wqt")
                wkt = L.tile([128, NC * 256], MMDT, tag="wkt")
                wvt = L.tile([128, NC * 256], MMDT, tag="wvt")
                for wtile, wdram in ((wqt, wq), (wkt, wk), (wvt, wv)):
                    nc.sync.dma_start(
                        wtile[:].rearrange("p (c d) -> p c d", d=256),
                        wdram.rearrange("(c p) d -> p c d", p=128),
                    )

                # QT2/KT2: [128 = head-pair d dims, T]; 4 accumulators in flight
                for dst, wtile in ((qt, wqt), (kt_, wkt)):
                    for grp in range(2):
                        outs = [(grp, tb) for tb in range(NQB)]
                        pss = [
                            PS1.tile([128, 512], F32, tag="ps1", name=f"p1_{id(wtile)}_{grp}_{tb}")
                            for tb in range(NQB)
                        ]
                        for c in range(NC):
                            for o, (p, tb) in enumerate(outs):
                                nc.tensor.matmul(
                                    pss[o][:],
                                    wtile[:, c * 256 + p * 128 : c * 256 + p * 128 + 128],
                                    xtb[:, c * T + tb * 512 : c * T + tb * 512 + 512],
                                    start=(c == 0),
                                    stop=(c == NC - 1),
                                )
                        for o, (p, tb) in enumerate(outs):
                            nc.vector.tensor_copy(
                                dst[p][:, tb * 512 : (tb + 1) * 512], pss[o][:]
                            )
                # V: natural layout [t, d], 4 heads side by side (65-strided groups)
                for tg in range(NT // 2):
                    pss = [
                        PSV.tile([128, 256], F32, tag="psv", name=f"pv_{tg}_{o}")
                        for o in range(2)
                    ]
                    for c in range(NC):
                        for o in range(2):
                            tt = 2 * tg + o
                            nc.tensor.matmul(
                                pss[o][:],
                                xtb[:, c * T + tt * 128 : c * T + tt * 128 + 128],
                                wvt[:, c * 256 : (c + 1) * 256],
                                start=(c == 0),
                                stop=(c == NC - 1),
                            )
                    for o in range(2):
                        tt = 2 * tg + o
                        dst = v4[:, tt * 260 : (tt + 1) * 260].rearrange(
                            "p (l j) -> p l j", j=65
                        )[:, :, 0:64]
                        nc.scalar.copy(dst, pss[o][:].rearrange("p (l j) -> p l j", j=64))

            # ---------------- phase 2: attention (+ fused output proj) ----------------
            # Software-pipelined: iteration N's attn@V + normalize are emitted
            # after iteration N+1's scores/exp so the PE never waits on ACT.
            with (
                tc.tile_pool(name="ph2", bufs=2) as E2,
                tc.tile_pool(name="stg", bufs=1) as STG,
                tc.tile_pool(name="nrm", bufs=2) as NP,
                tc.tile_pool(name="ph3", bufs=3) as S3,
                tc.tile_pool(name="ps2", bufs=1, space="PSUM") as PS2,
                tc.tile_pool(name="pso", bufs=2, space="PSUM") as PSO,
            ):

                def emit_scores(qb, p):
                    e01 = [
                        E2.tile([128, NT * 512], MMDT, tag=f"e{l}", name=f"e{l}_{p}_{qb}")
                        for l in range(2)
                    ]
                    qs = qt[p][:, qb * 512 : (qb + 1) * 512]
                    for ktp in range(8):
                        sAB = [
                            PS2.tile([128, 1024], F32, tag=f"s{l}", name=f"s{l}_{p}_{qb}_{ktp}")
                            for l in range(2)
                        ]
                        for half in range(2):
                            kti = 2 * ktp + half
                            for l in range(2):
                                lo, hi = 64 * l, 64 * l + 64
                                nc.tensor.matmul(
                                    sAB[l][:, half * 512 : half * 512 + 512],
                                    kt_[p][lo:hi, kti * 128 : kti * 128 + 128],
                                    qs[lo:hi, :],
                                    start=True,
                                    stop=True,
                                )
                        for l in range(2):
                            nc.scalar.activation(
                                e01[l][:, ktp * 1024 : ktp * 1024 + 1024],
                                sAB[l][:],
                                EXP,
                                scale=SCALE,
                            )
                    return e01

                def emit_tail(qb, p, e01):
                    po = [
                        PSO.tile([65, 512], F32, tag=f"po{l}", name=f"po{l}_{p}_{qb}")
                        for l in range(2)
                    ]
                    for kti in range(NT):
                        for l in range(2):
                            g0 = kti * 260 + (2 * p + l) * 65
                            nc.tensor.matmul(
                                po[l][:],
                                v4[:, g0 : g0 + 65],
                                e01[l][:, kti * 512 : kti * 512 + 512],
                                start=(kti == 0),
                                stop=(kti == NT - 1),
                                skip_group_check=True,
                            )
                    for l in range(2):
                        e = e01[l]
                        srow = NP.tile([1, 512], F32, tag="srow")
                        nc.vector.tensor_copy(srow[:], po[l][64:65, :])
                        rbraw = NP.tile([128, 1024], F32, tag="rbraw")
                        nc.gpsimd.partition_broadcast(rbraw[:, 0:512], srow[:])
                        nc.gpsimd.partition_broadcast(rbraw[:, 512:1024], srow[:])
                        rb2 = NP.tile([128, 1024], F32, tag="rb2")
                        nc.vector.reciprocal(rb2[:], rbraw[:])
                        if l == 0:
                            nc.vector.tensor_tensor(
                                ot2[p][0:64, qb * 512 : qb * 512 + 512],
                                po[l][0:64, :],
                                rb2[0:64, 0:512],
                                op=mybir.AluOpType.mult,
                            )
                        else:
                            tmp = NP.tile([64, 512], MMDT, tag="tmp")
                            nc.vector.tensor_tensor(
                                tmp[:],
                                po[l][0:64, :],
                                rb2[0:64, 0:512],
                                op=mybir.AluOpType.mult,
                            )
                            nc.sync.dma_start(
                                ot2[p][64:128, qb * 512 : qb * 512 + 512], tmp[:]
                            )
                        lg = 2 * p + l
                        for hf in range(2):
                            stage = STG.tile(
                                [128, 8 * 512], F32, tag=f"st{l}{hf}",
                                name=f"st{l}{hf}_{p}_{qb}",
                            )
                            for kp in range(4):
                                eng = nc.gpsimd if kp == 3 else nc.vector
                                eng.tensor_tensor(
                                    stage[:, kp * 1024 : kp * 1024 + 1024],
                                    e[:, hf * 4096 + kp * 1024 : hf * 4096 + kp * 1024 + 1024],
                                    rb2[:],
                                    op=mybir.AluOpType.mult,
                                )
                            nc.sync.dma_start(
                                attnT[lg].rearrange("(kt p) q -> p kt q", p=128)[
                                    :, hf * 8 : hf * 8 + 8, qb * 512 : qb * 512 + 512
                                ],
                                stage[:].rearrange("p (kt j) -> p kt j", j=512),
                            )
                    if p == 1:
                        for tt in range(4 * qb, 4 * qb + 4):
                            for db in range(2):
                                ps = PS2.tile([128, 512], F32, tag="s0", name=f"p3_{tt}_{db}")
                                for pp in range(2):
                                    nc.tensor.matmul(
                                        ps[:],
                                        ot2[pp][:, tt * 128 : tt * 128 + 128],
                                        wot[:, pp * D + db * 512 : pp * D + db * 512 + 512],
                                        start=(pp == 0),
                                        stop=(pp == 1),
                                    )
                                st = S3.tile([128, 512], F32, tag="st")
                                nc.vector.tensor_copy(st[:], ps[:])
                                nc.sync.dma_start(
                                    outp[tt * 128 : tt * 128 + 128, db * 512 : db * 512 + 512],
                                    st[:],
                                )

                pending = None
                for qb in range(NQB):
                    for p in range(2):
                        e01 = emit_scores(qb, p)
                        if pending is not None:
                            emit_tail(*pending)
                        pending = (qb, p, e01)
                emit_tail(*pending)

    nc.compile()
    return nc


def kernel(x, W_q, W_k, W_v, W_o):
    global LAST_RESULT
    x = np.asarray(x, dtype=np.float32)
    W_q = np.asarray(W_q, dtype=np.float32)
    W_k = np.asarray(W_k, dtype=np.float32)
    W_v = np.asarray(W_v, dtype=np.float32)
    W_o = np.asarray(W_o, dtype=np.float32)

    nc = _build()

    ones = np.ones((128, 64), dtype=NPDT)
    in_maps = []
    for c in range(8):
        b = c // 4
        hs = 4 * (c % 4)  # first global head of this core
        r0 = 64 * hs
        in_maps.append(
            {
                "xT": np.ascontiguousarray(x[b].T).astype(NPDT),
                "wq": np.ascontiguousarray(W_q[r0 : r0 + 256, :].T).astype(NPDT),
                "wk": np.ascontiguousarray(W_k[r0 : r0 + 256, :].T).astype(NPDT),
                "wv": np.ascontiguousarray(W_v[r0 : r0 + 256, :].T).astype(NPDT),
                "wo": np.ascontiguousarray(W_o[:, r0 : r0 + 256].T).astype(NPDT),
                "ones": ones,
            }
        )

    res = run_bass_kernel_spmd(nc, in_maps, core_ids=list(range(8)))
    LAST_RESULT = res

    B, H = 2, 16
    out = np.zeros((B, T, D), dtype=np.float32)
    attn = np.empty((B, H, T, T), dtype=np.float32)
    for c in range(8):
        b = c // 4
        hs = 4 * (c % 4)
        r = res.results[c]
        out[b] += r["outp"]
        a = r["attnT"]  # [4, k, q]
        for l in range(HL):
            attn[b, hs + l] = a[l].T
    return out, attn


# revision 13
# speedup vs baseline: 1.1359x; 1.0625x over previous
"""Multi-head attention (B=2, T=2048, D=1024, H=16) on 8 TRN2 NeuronCores.

Sharding: core c handles batch b = c//4 and heads 4*(c%4) .. 4*(c%4)+3.
Each core computes Q/K/V projections for its 4 heads, attention scores in
transposed layout (scoresT[k, q]), softmax via exp + a ones-column appended to
V (so the denominator falls out of the attn@V matmul), the attention-weighted
values, and a partial output projection. The host assembles the attention
weights (transpose of the device layout) and sums the 4 per-core W_o partials
per batch.

Matmuls run in fp16 (10-bit mantissa, full PE rate); the projections
accumulate in fp32 PSUM; exp on ScalarE; normalization in fp32.
"""
import numpy as np

import concourse.bacc as bacc
import concourse.mybir as mybir
from concourse import tile
from concourse.bass_utils import run_bass_kernel_spmd

T = 2048
D = 1024
HL = 4  # heads per core
NC = 8  # D / 128 contraction chunks
NT = 16  # T / 128 tiles (t-tiles and k-tiles)
NQB = 4  # T / 512 query blocks
F32 = mybir.dt.float32
MMDT = mybir.dt.float16  # matmul operand dtype
NPDT = np.float16
EXP = mybir.ActivationFunctionType.Exp
SCALE = 0.125  # 1/sqrt(d_k)

LAST_RESULT = None  # stashed BassKernelResults for test harnesses


def _build():
    nc = bacc.Bacc("TRN2", target_bir_lowering=False, debug=False, num_devices=8)

    xT = nc.dram_tensor("xT", [D, T], MMDT, kind="ExternalInput").ap()
    wq = nc.dram_tensor("wq", [D, 256], MMDT, kind="ExternalInput").ap()
    wk = nc.dram_tensor("wk", [D, 256], MMDT, kind="ExternalInput").ap()
    wv = nc.dram_tensor("wv", [D, 256], MMDT, kind="ExternalInput").ap()
    wo = nc.dram_tensor("wo", [256, D], MMDT, kind="ExternalInput").ap()
    onesd = nc.dram_tensor("ones", [128, 64], MMDT, kind="ExternalInput").ap()
    attnT = nc.dram_tensor("attnT", [HL, T, T], F32, kind="ExternalOutput").ap()
    outp = nc.dram_tensor("outp", [T, D], F32, kind="ExternalOutput").ap()

    with tile.TileContext(nc) as tc:
        with tc.tile_pool(name="persist", bufs=1) as P:
            qt = [P.tile([128, T], MMDT, tag=f"qt{p}", name=f"qt{p}") for p in range(2)]
            kt_ = [P.tile([128, T], MMDT, tag=f"kt{p}", name=f"kt{p}") for p in range(2)]
            v4 = P.tile([128, NT * 260], MMDT, tag="v4")  # [V_l | 1] per (tt, l)
            ot2 = [P.tile([128, T], MMDT, tag=f"ot{p}", name=f"ot{p}") for p in range(2)]
            wot = P.tile([128, 2 * D], MMDT, tag="wot")

            # ones columns of v4: position tt*260 + l*65 + 64
            v4g = v4[:].rearrange("p (g j) -> p g j", j=65)  # g = tt*4 + l
            nc.sync.dma_start(v4g[:, :, 64:65], onesd.unsqueeze(2))
            nc.sync.dma_start(
                wot[:].rearrange("p (c t) -> p c t", t=D),
                wo.rearrange("(c p) t -> p c t", p=128),
            )

            # ---------------- phase 1: load x, project Q/K/V ----------------
            with (
                tc.tile_pool(name="ph1", bufs=1) as L,
                tc.tile_pool(name="ps1", bufs=4, space="PSUM") as PS1,
                tc.tile_pool(name="psv", bufs=2, space="PSUM") as PSV,
            ):
                xtb = L.tile([128, NC * T], MMDT, tag="xtb")
                wqt = L.tile([128, NC * 256], MMDT, tag="wqt")
                wkt = L.tile([128, NC * 256], MMDT, tag="wkt")
                wvt = L.tile([128, NC * 256], MMDT, tag="wvt")
                for wtile, wdram in ((wqt, wq), (wkt, wk), (wvt, wv)):
                    nc.sync.dma_start(
                        wtile[:].rearrange("p (c d) -> p c d", d=256),
                        wdram.rearrange("(c p) d -> p c d", p=128),
                    )
                for c in range(NC):
                    nc.sync.dma_start(
                        xtb[:, c * T : (c + 1) * T], xT[c * 128 : (c + 1) * 128, :]
                    )

                # QT2/KT2: [128 = head-pair d dims, T]; 4 accumulators in flight
                for dst, wtile in ((qt, wqt), (kt_, wkt)):
                    for grp in range(2):
                        outs = [(grp, tb) for tb in range(NQB)]
                        pss = [
                            PS1.tile([128, 512], F32, tag="ps1", name=f"p1_{id(wtile)}_{grp}_{tb}")
                            for tb in range(NQB)
                        ]
                        for c in range(NC):
                            for o, (p, tb) in enumerate(outs):
                                nc.tensor.matmul(
                                    pss[o][:],
                                    wtile[:, c * 256 + p * 128 : c * 256 + p * 128 + 128],
                                    xtb[:, c * T + tb * 512 : c * T + tb * 512 + 512],
                                    start=(c == 0),
                                    stop=(c == NC - 1),
                                )
                        for o, (p, tb) in enumerate(outs):
                            nc.vector.tensor_copy(
                                dst[p][:, tb * 512 : (tb + 1) * 512], pss[o][:]
                            )
                # V: natural layout [t, d], 4 heads side by side (65-strided groups)
                for tg in range(NT // 2):
                    pss = [
                        PSV.tile([128, 256], F32, tag="psv", name=f"pv_{tg}_{o}")
                        for o in range(2)
                    ]
                    for c in range(NC):
                        for o in range(2):
                            tt = 2 * tg + o
                            nc.tensor.matmul(
                                pss[o][:],
                                xtb[:, c * T + tt * 128 : c * T + tt * 128 + 128],
                                wvt[:, c * 256 : (c + 1) * 256],
                                start=(c == 0),
                                stop=(c == NC - 1),
                            )
                    for o in range(2):
                        tt = 2 * tg + o
                        dst = v4[:, tt * 260 : (tt + 1) * 260].rearrange(
                            "p (l j) -> p l j", j=65
                        )[:, :, 0:64]
                        nc.scalar.copy(dst, pss[o][:].rearrange("p (l j) -> p l j", j=64))

            # ---------------- phase 2: attention (+ fused output proj) ----------------
            # Software-pipelined: iteration N's attn@V + normalize are emitted
            # after iteration N+1's scores/exp so the PE never waits on ACT.
            with (
                tc.tile_pool(name="ph2", bufs=3) as E2,
                tc.tile_pool(name="stg", bufs=1) as STG,
                tc.tile_pool(name="nrm", bufs=2) as NP,
                tc.tile_pool(name="ph3", bufs=3) as S3,
                tc.tile_pool(name="ps2", bufs=1, space="PSUM") as PS2,
                tc.tile_pool(name="pso", bufs=2, space="PSUM") as PSO,
            ):

                def emit_scores(qb, p):
                    e01 = [
                        E2.tile([128, NT * 512], MMDT, tag=f"e{l}", name=f"e{l}_{p}_{qb}")
                        for l in range(2)
                    ]
                    qs = qt[p][:, qb * 512 : (qb + 1) * 512]
                    for ktp in range(8):
                        sAB = [
                            PS2.tile([128, 1024], F32, tag=f"s{l}", name=f"s{l}_{p}_{qb}_{ktp}")
                            for l in range(2)
                        ]
                        for half in range(2):
                            kti = 2 * ktp + half
                            for l in range(2):
                                lo, hi = 64 * l, 64 * l + 64
                                nc.tensor.matmul(
                                    sAB[l][:, half * 512 : half * 512 + 512],
                                    kt_[p][lo:hi, kti * 128 : kti * 128 + 128],
                                    qs[lo:hi, :],
                                    start=True,
                                    stop=True,
                                )
                        for l in range(2):
                            nc.scalar.activation(
                                e01[l][:, ktp * 1024 : ktp * 1024 + 1024],
                                sAB[l][:],
                                EXP,
                                scale=SCALE,
                            )
                    return e01

                def emit_tail(qb, p, e01):
                    po = [
                        PSO.tile([65, 512], F32, tag=f"po{l}", name=f"po{l}_{p}_{qb}")
                        for l in range(2)
                    ]
                    for kti in range(NT):
                        for l in range(2):
                            g0 = kti * 260 + (2 * p + l) * 65
                            nc.tensor.matmul(
                                po[l][:],
                                v4[:, g0 : g0 + 65],
                                e01[l][:, kti * 512 : kti * 512 + 512],
                                start=(kti == 0),
                                stop=(kti == NT - 1),
                                skip_group_check=True,
                            )
                    for l in range(2):
                        e = e01[l]
                        srow = NP.tile([1, 512], F32, tag="srow")
                        nc.vector.tensor_copy(srow[:], po[l][64:65, :])
                        rbraw = NP.tile([128, 1024], F32, tag="rbraw")
                        nc.gpsimd.partition_broadcast(rbraw[:, 0:512], srow[:])
                        nc.gpsimd.partition_broadcast(rbraw[:, 512:1024], srow[:])
                        rb2 = NP.tile([128, 1024], F32, tag="rb2")
                        nc.vector.reciprocal(rb2[:], rbraw[:])
                        if l == 0:
                            nc.vector.tensor_tensor(
                                ot2[p][0:64, qb * 512 : qb * 512 + 512],
                                po[l][0:64, :],
                                rb2[0:64, 0:512],
                                op=mybir.AluOpType.mult,
                            )
                        else:
                            tmp = NP.tile([64, 512], MMDT, tag="tmp")
                            nc.vector.tensor_tensor(
                                tmp[:],
                                po[l][0:64, :],
                                rb2[0:64, 0:512],
                                op=mybir.AluOpType.mult,
                            )
                            nc.sync.dma_start(
                                ot2[p][64:128, qb * 512 : qb * 512 + 512], tmp[:]
                            )
                        lg = 2 * p + l
                        for hf in range(2):
                            stage = STG.tile(
                                [128, 8 * 512], F32, tag=f"st{l}",
                                name=f"st{l}{hf}_{p}_{qb}",
                            )
                            for kp in range(4):
                                eng = nc.gpsimd if kp == 3 else nc.vector
                                eng.tensor_tensor(
                                    stage[:, kp * 1024 : kp * 1024 + 1024],
                                    e[:, hf * 4096 + kp * 1024 : hf * 4096 + kp * 1024 + 1024],
                                    rb2[:],
                                    op=mybir.AluOpType.mult,
                                )
                            nc.sync.dma_start(
                                attnT[lg].rearrange("(kt p) q -> p kt q", p=128)[
                                    :, hf * 8 : hf * 8 + 8, qb * 512 : qb * 512 + 512
                                ],
                                stage[:].rearrange("p (kt j) -> p kt j", j=512),
                            )
                    if p == 1:
                        for tt in range(4 * qb, 4 * qb + 4):
                            for db in range(2):
                                ps = PS2.tile([128, 512], F32, tag="s0", name=f"p3_{tt}_{db}")
                                for pp in range(2):
                                    nc.tensor.matmul(
                                        ps[:],
                                        ot2[pp][:, tt * 128 : tt * 128 + 128],
                                        wot[:, pp * D + db * 512 : pp * D + db * 512 + 512],
                                        start=(pp == 0),
                                        stop=(pp == 1),
                                    )
                                st = S3.tile([128, 512], F32, tag="st")
                                nc.vector.tensor_copy(st[:], ps[:])
                                nc.sync.dma_start(
                                    outp[tt * 128 : tt * 128 + 128, db * 512 : db * 512 + 512],
                                    st[:],
                                )

                pending = None
                for qb in range(NQB):
                    for p in range(2):
                        e01 = emit_scores(qb, p)
                        if pending is not None:
                            emit_tail(*pending)
                        pending = (qb, p, e01)
                emit_tail(*pending)

    nc.compile()
    return nc


def kernel(x, W_q, W_k, W_v, W_o):
    global LAST_RESULT
    x = np.asarray(x, dtype=np.float32)
    W_q = np.asarray(W_q, dtype=np.float32)
    W_k = np.asarray(W_k, dtype=np.float32)
    W_v = np.asarray(W_v, dtype=np.float32)
    W_o = np.asarray(W_o, dtype=np.float32)

    nc = _build()

    ones = np.ones((128, 64), dtype=NPDT)
    in_maps = []
    for c in range(8):
        b = c // 4
        hs = 4 * (c % 4)  # first global head of this core
        r0 = 64 * hs
        in_maps.append(
            {
                "xT": np.ascontiguousarray(x[b].T).astype(NPDT),
                "wq": np.ascontiguousarray(W_q[r0 : r0 + 256, :].T).astype(NPDT),
                "wk": np.ascontiguousarray(W_k[r0 : r0 + 256, :].T).astype(NPDT),
                "wv": np.ascontiguousarray(W_v[r0 : r0 + 256, :].T).astype(NPDT),
                "wo": np.ascontiguousarray(W_o[:, r0 : r0 + 256].T).astype(NPDT),
                "ones": ones,
            }
        )

    res = run_bass_kernel_spmd(nc, in_maps, core_ids=list(range(8)))
    LAST_RESULT = res

    B, H = 2, 16
    out = np.zeros((B, T, D), dtype=np.float32)
    attn = np.empty((B, H, T, T), dtype=np.float32)
    for c in range(8):
        b = c // 4
        hs = 4 * (c % 4)
        r = res.results[c]
        out[b] += r["outp"]
        a = r["attnT"]  # [4, k, q]
        for l in range(HL):
            attn[b, hs + l] = a[l].T
    return out, attn


# revision 14
# speedup vs baseline: 1.1369x; 1.0009x over previous
"""Multi-head attention (B=2, T=2048, D=1024, H=16) on 8 TRN2 NeuronCores.

Sharding: core c handles batch b = c//4 and heads 4*(c%4) .. 4*(c%4)+3.
Each core computes Q/K/V projections for its 4 heads, attention scores in
transposed layout (scoresT[k, q]), softmax via exp + a ones-column appended to
V (so the denominator falls out of the attn@V matmul), the attention-weighted
values, and a partial output projection. The host assembles the attention
weights (transpose of the device layout) and sums the 4 per-core W_o partials
per batch.

Matmuls run in fp16 (10-bit mantissa, full PE rate); the projections
accumulate in fp32 PSUM; exp on ScalarE; normalization in fp32.
"""
import numpy as np

import concourse.bacc as bacc
import concourse.mybir as mybir
from concourse import tile
from concourse.bass_utils import run_bass_kernel_spmd

T = 2048
D = 1024
HL = 4  # heads per core
NC = 8  # D / 128 contraction chunks
NT = 16  # T / 128 tiles (t-tiles and k-tiles)
NQB = 4  # T / 512 query blocks
F32 = mybir.dt.float32
MMDT = mybir.dt.float16  # matmul operand dtype
NPDT = np.float16
EXP = mybir.ActivationFunctionType.Exp
SCALE = 0.125  # 1/sqrt(d_k)

LAST_RESULT = None  # stashed BassKernelResults for test harnesses


def _build():
    nc = bacc.Bacc("TRN2", target_bir_lowering=False, debug=False, num_devices=8)

    xT = nc.dram_tensor("xT", [D, T], MMDT, kind="ExternalInput").ap()
    wq = nc.dram_tensor("wq", [D, 256], MMDT, kind="ExternalInput").ap()
    wk = nc.dram_tensor("wk", [D, 256], MMDT, kind="ExternalInput").ap()
    wv = nc.dram_tensor("wv", [D, 256], MMDT, kind="ExternalInput").ap()
    wo = nc.dram_tensor("wo", [256, D], MMDT, kind="ExternalInput").ap()
    onesd = nc.dram_tensor("ones", [128, 64], MMDT, kind="ExternalInput").ap()
    attnT = nc.dram_tensor("attnT", [HL, T, T], F32, kind="ExternalOutput").ap()
    outp = nc.dram_tensor("outp", [T, D], F32, kind="ExternalOutput").ap()

    with tile.TileContext(nc) as tc:
        with tc.tile_pool(name="persist", bufs=1) as P:
            qt = [P.tile([128, T], MMDT, tag=f"qt{p}", name=f"qt{p}") for p in range(2)]
            kt_ = [P.tile([128, T], MMDT, tag=f"kt{p}", name=f"kt{p}") for p in range(2)]
            v4 = P.tile([128, NT * 260], MMDT, tag="v4")  # [V_l | 1] per (tt, l)
            ot2 = [P.tile([128, T], MMDT, tag=f"ot{p}", name=f"ot{p}") for p in range(2)]
            wot = P.tile([128, 2 * D], MMDT, tag="wot")

            # ones columns of v4: position tt*260 + l*65 + 64
            v4g = v4[:].rearrange("p (g j) -> p g j", j=65)  # g = tt*4 + l
            nc.sync.dma_start(v4g[:, :, 64:65], onesd.unsqueeze(2))
            nc.sync.dma_start(
                wot[:].rearrange("p (c t) -> p c t", t=D),
                wo.rearrange("(c p) t -> p c t", p=128),
            )

            # ---------------- phase 1: load x, project Q/K/V ----------------
            with (
                tc.tile_pool(name="ph1", bufs=1) as L,
                tc.tile_pool(name="ps1", bufs=4, space="PSUM") as PS1,
                tc.tile_pool(name="psv", bufs=2, space="PSUM") as PSV,
            ):
                xtb = L.tile([128, NC * T], MMDT, tag="xtb")
                wqt = L.tile([128, NC * 256], MMDT, tag="wqt")
                wkt = L.tile([128, NC * 256], MMDT, tag="wkt")
                wvt = L.tile([128, NC * 256], MMDT, tag="wvt")
                for wtile, wdram in ((wqt, wq), (wkt, wk), (wvt, wv)):
                    nc.sync.dma_start(
                        wtile[:].rearrange("p (c d) -> p c d", d=256),
                        wdram.rearrange("(c p) d -> p c d", p=128),
                    )
                for c in range(NC):
                    nc.sync.dma_start(
                        xtb[:, c * T : (c + 1) * T], xT[c * 128 : (c + 1) * 128, :]
                    )

                # QT2/KT2: [128 = head-pair d dims, T]; 4 accumulators in flight
                for dst, wtile in ((qt, wqt), (kt_, wkt)):
                    for grp in range(2):
                        outs = [(grp, tb) for tb in range(NQB)]
                        pss = [
                            PS1.tile([128, 512], F32, tag="ps1", name=f"p1_{id(wtile)}_{grp}_{tb}")
                            for tb in range(NQB)
                        ]
                        for c in range(NC):
                            for o, (p, tb) in enumerate(outs):
                                nc.tensor.matmul(
                                    pss[o][:],
                                    wtile[:, c * 256 + p * 128 : c * 256 + p * 128 + 128],
                                    xtb[:, c * T + tb * 512 : c * T + tb * 512 + 512],
                                    start=(c == 0),
                                    stop=(c == NC - 1),
                                )
                        for o, (p, tb) in enumerate(outs):
                            nc.vector.tensor_copy(
                                dst[p][:, tb * 512 : (tb + 1) * 512], pss[o][:]
                            )
                # V: natural layout [t, d], 4 heads side by side (65-strided groups)
                for tg in range(NT // 2):
                    pss = [
                        PSV.tile([128, 256], F32, tag="psv", name=f"pv_{tg}_{o}")
                        for o in range(2)
                    ]
                    for c in range(NC):
                        for o in range(2):
                            tt = 2 * tg + o
                            nc.tensor.matmul(
                                pss[o][:],
                                xtb[:, c * T + tt * 128 : c * T + tt * 128 + 128],
                                wvt[:, c * 256 : (c + 1) * 256],
                                start=(c == 0),
                                stop=(c == NC - 1),
                            )
                    for o in range(2):
                        tt = 2 * tg + o
                        dst = v4[:, tt * 260 : (tt + 1) * 260].rearrange(
                            "p (l j) -> p l j", j=65
                        )[:, :, 0:64]
                        nc.scalar.copy(dst, pss[o][:].rearrange("p (l j) -> p l j", j=64))

            # ---------------- phase 2: attention (+ fused output proj) ----------------
            # Software-pipelined: iteration N's attn@V + normalize are emitted
            # after iteration N+1's scores/exp so the PE never waits on ACT.
            with (
                tc.tile_pool(name="ph2", bufs=3) as E2,
                tc.tile_pool(name="stg", bufs=1) as STG,
                tc.tile_pool(name="nrm", bufs=2) as NP,
                tc.tile_pool(name="ph3", bufs=3) as S3,
                tc.tile_pool(name="ps2", bufs=3, space="PSUM") as PS2,
                tc.tile_pool(name="pso", bufs=1, space="PSUM") as PSO,
            ):

                def emit_scores(qb, p):
                    e01 = [
                        E2.tile([128, NT * 512], MMDT, tag=f"e{l}", name=f"e{l}_{p}_{qb}")
                        for l in range(2)
                    ]
                    qs = qt[p][:, qb * 512 : (qb + 1) * 512]
                    for ktp in range(8):
                        sAB = [
                            PS2.tile([128, 1024], F32, tag="s", name=f"s{l}_{p}_{qb}_{ktp}")
                            for l in range(2)
                        ]
                        for half in range(2):
                            kti = 2 * ktp + half
                            for l in range(2):
                                lo, hi = 64 * l, 64 * l + 64
                                nc.tensor.matmul(
                                    sAB[l][:, half * 512 : half * 512 + 512],
                                    kt_[p][lo:hi, kti * 128 : kti * 128 + 128],
                                    qs[lo:hi, :],
                                    start=True,
                                    stop=True,
                                )
                        for l in range(2):
                            nc.scalar.activation(
                                e01[l][:, ktp * 1024 : ktp * 1024 + 1024],
                                sAB[l][:],
                                EXP,
                                scale=SCALE,
                            )
                    return e01

                def emit_tail(qb, p, e01):
                    po = [
                        PSO.tile([65, 512], F32, tag=f"po{l}", name=f"po{l}_{p}_{qb}")
                        for l in range(2)
                    ]
                    for kti in range(NT):
                        for l in range(2):
                            g0 = kti * 260 + (2 * p + l) * 65
                            nc.tensor.matmul(
                                po[l][:],
                                v4[:, g0 : g0 + 65],
                                e01[l][:, kti * 512 : kti * 512 + 512],
                                start=(kti == 0),
                                stop=(kti == NT - 1),
                                skip_group_check=True,
                            )
                    for l in range(2):
                        e = e01[l]
                        srow = NP.tile([1, 512], F32, tag="srow")
                        nc.vector.tensor_copy(srow[:], po[l][64:65, :])
                        rbraw = NP.tile([128, 1024], F32, tag="rbraw")
                        nc.gpsimd.partition_broadcast(rbraw[:, 0:512], srow[:])
                        nc.gpsimd.partition_broadcast(rbraw[:, 512:1024], srow[:])
                        rb2 = NP.tile([128, 1024], F32, tag="rb2")
                        nc.vector.reciprocal(rb2[:], rbraw[:])
                        if l == 0:
                            nc.vector.tensor_tensor(
                                ot2[p][0:64, qb * 512 : qb * 512 + 512],
                                po[l][0:64, :],
                                rb2[0:64, 0:512],
                                op=mybir.AluOpType.mult,
                            )
                        else:
                            tmp = NP.tile([64, 512], MMDT, tag="tmp")
                            nc.vector.tensor_tensor(
                                tmp[:],
                                po[l][0:64, :],
                                rb2[0:64, 0:512],
                                op=mybir.AluOpType.mult,
                            )
                            nc.sync.dma_start(
                                ot2[p][64:128, qb * 512 : qb * 512 + 512], tmp[:]
                            )
                        lg = 2 * p + l
                        for hf in range(2):
                            stage = STG.tile(
                                [128, 8 * 512], F32, tag=f"st{l}",
                                name=f"st{l}{hf}_{p}_{qb}",
                            )
                            for kp in range(4):
                                eng = nc.gpsimd if kp >= 2 else nc.vector
                                eng.tensor_tensor(
                                    stage[:, kp * 1024 : kp * 1024 + 1024],
                                    e[:, hf * 4096 + kp * 1024 : hf * 4096 + kp * 1024 + 1024],
                                    rb2[:],
                                    op=mybir.AluOpType.mult,
                                )
                            nc.sync.dma_start(
                                attnT[lg].rearrange("(kt p) q -> p kt q", p=128)[
                                    :, hf * 8 : hf * 8 + 8, qb * 512 : qb * 512 + 512
                                ],
                                stage[:].rearrange("p (kt j) -> p kt j", j=512),
                            )
                    if p == 1:
                        for tt in range(4 * qb, 4 * qb + 4):
                            for db in range(2):
                                ps = PS2.tile([128, 512], F32, tag="s", name=f"p3_{tt}_{db}")
                                for pp in range(2):
                                    nc.tensor.matmul(
                                        ps[:],
                                        ot2[pp][:, tt * 128 : tt * 128 + 128],
                                        wot[:, pp * D + db * 512 : pp * D + db * 512 + 512],
                                        start=(pp == 0),
                                        stop=(pp == 1),
                                    )
                                st = S3.tile([128, 512], F32, tag="st")
                                nc.vector.tensor_copy(st[:], ps[:])
                                nc.sync.dma_start(
                                    outp[tt * 128 : tt * 128 + 128, db * 512 : db * 512 + 512],
                                    st[:],
                                )

                pending = None
                for qb in range(NQB):
                    for p in range(2):
                        e01 = emit_scores(qb, p)
                        if pending is not None:
                            emit_tail(*pending)
                        pending = (qb, p, e01)
                emit_tail(*pending)

    nc.compile()
    return nc


def kernel(x, W_q, W_k, W_v, W_o):
    global LAST_RESULT
    x = np.asarray(x, dtype=np.float32)
    W_q = np.asarray(W_q, dtype=np.float32)
    W_k = np.asarray(W_k, dtype=np.float32)
    W_v = np.asarray(W_v, dtype=np.float32)
    W_o = np.asarray(W_o, dtype=np.float32)

    nc = _build()

    ones = np.ones((128, 64), dtype=NPDT)
    in_maps = []
    for c in range(8):
        b = c // 4
        hs = 4 * (c % 4)  # first global head of this core
        r0 = 64 * hs
        in_maps.append(
            {
                "xT": np.ascontiguousarray(x[b].T).astype(NPDT),
                "wq": np.ascontiguousarray(W_q[r0 : r0 + 256, :].T).astype(NPDT),
                "wk": np.ascontiguousarray(W_k[r0 : r0 + 256, :].T).astype(NPDT),
                "wv": np.ascontiguousarray(W_v[r0 : r0 + 256, :].T).astype(NPDT),
                "wo": np.ascontiguousarray(W_o[:, r0 : r0 + 256].T).astype(NPDT),
                "ones": ones,
            }
        )

    res = run_bass_kernel_spmd(nc, in_maps, core_ids=list(range(8)))
    LAST_RESULT = res

    B, H = 2, 16
    out = np.zeros((B, T, D), dtype=np.float32)
    attn = np.empty((B, H, T, T), dtype=np.float32)
    for c in range(8):
        b = c // 4
        hs = 4 * (c % 4)
        r = res.results[c]
        out[b] += r["outp"]
        a = r["attnT"]  # [4, k, q]
        for l in range(HL):
            attn[b, hs + l] = a[l].T
    return out, attn


# revision 16
# speedup vs baseline: 1.3346x; 1.1738x over previous
"""Multi-head attention (B=2, T=2048, D=1024, H=16) on 8 TRN2 NeuronCores.

Sharding: core c handles batch b = c//4 and heads 4*(c%4) .. 4*(c%4)+3.
Each core computes Q/K/V projections for its 4 heads, attention scores in
transposed layout (scoresT[k, q]), softmax via exp + a ones-column appended to
V (so the denominator falls out of the attn@V matmul), the attention-weighted
values, and a partial output projection. The host assembles the attention
weights (transpose of the device layout) and sums the 4 per-core W_o partials
per batch.

Matmuls run in fp16 (10-bit mantissa, full PE rate); the projections
accumulate in fp32 PSUM; exp on ScalarE; normalization in fp32.
"""
import numpy as np

import concourse.bacc as bacc
import concourse.mybir as mybir
from concourse import tile
from concourse.bass_utils import run_bass_kernel_spmd

T = 2048
D = 1024
HL = 4  # heads per core
NC = 8  # D / 128 contraction chunks
NT = 16  # T / 128 tiles (t-tiles and k-tiles)
NQB = 4  # T / 512 query blocks
F32 = mybir.dt.float32
MMDT = mybir.dt.float16  # matmul operand dtype
NPDT = np.float16
EXP = mybir.ActivationFunctionType.Exp
SCALE = 0.125  # 1/sqrt(d_k)

LAST_RESULT = None  # stashed BassKernelResults for test harnesses


def _build():
    nc = bacc.Bacc("TRN2", target_bir_lowering=False, debug=False, num_devices=8)

    xT = nc.dram_tensor("xT", [D, T], MMDT, kind="ExternalInput").ap()
    wq = nc.dram_tensor("wq", [D, 256], MMDT, kind="ExternalInput").ap()
    wk = nc.dram_tensor("wk", [D, 256], MMDT, kind="ExternalInput").ap()
    wv = nc.dram_tensor("wv", [D, 256], MMDT, kind="ExternalInput").ap()
    wo = nc.dram_tensor("wo", [256, D], MMDT, kind="ExternalInput").ap()
    onesd = nc.dram_tensor("ones", [128, 64], MMDT, kind="ExternalInput").ap()
    onesr = nc.dram_tensor("onesr", [1, 128], MMDT, kind="ExternalInput").ap()
    attnT = nc.dram_tensor("attnT", [HL, T, T], F32, kind="ExternalOutput").ap()
    outp = nc.dram_tensor("outp", [T, D], F32, kind="ExternalOutput").ap()

    with tile.TileContext(nc) as tc:
        with tc.tile_pool(name="persist", bufs=1) as P:
            qt = [P.tile([128, T], MMDT, tag=f"qt{p}", name=f"qt{p}") for p in range(2)]
            kt_ = [P.tile([128, T], MMDT, tag=f"kt{p}", name=f"kt{p}") for p in range(2)]
            v4 = P.tile([128, NT * 260], MMDT, tag="v4")  # [V_l | 1] per (tt, l)
            ot2 = [P.tile([128, T], MMDT, tag=f"ot{p}", name=f"ot{p}") for p in range(2)]
            wot = P.tile([128, 2 * D], MMDT, tag="wot")
            onesrt = P.tile([1, 128], MMDT, tag="onesrt")
            nc.sync.dma_start(onesrt[:], onesr[:])

            # ones columns of v4: position tt*260 + l*65 + 64
            v4g = v4[:].rearrange("p (g j) -> p g j", j=65)  # g = tt*4 + l
            nc.sync.dma_start(v4g[:, :, 64:65], onesd.unsqueeze(2))
            nc.sync.dma_start(
                wot[:].rearrange("p (c t) -> p c t", t=D),
                wo.rearrange("(c p) t -> p c t", p=128),
            )

            # ---------------- phase 1: load x, project Q/K/V ----------------
            with (
                tc.tile_pool(name="ph1", bufs=1) as L,
                tc.tile_pool(name="ps1", bufs=4, space="PSUM") as PS1,
                tc.tile_pool(name="psv", bufs=2, space="PSUM") as PSV,
            ):
                xtb = L.tile([128, NC * T], MMDT, tag="xtb")
                wqt = L.tile([128, NC * 256], MMDT, tag="wqt")
                wkt = L.tile([128, NC * 256], MMDT, tag="wkt")
                wvt = L.tile([128, NC * 256], MMDT, tag="wvt")
                for wtile, wdram in ((wqt, wq), (wkt, wk), (wvt, wv)):
                    nc.sync.dma_start(
                        wtile[:].rearrange("p (c d) -> p c d", d=256),
                        wdram.rearrange("(c p) d -> p c d", p=128),
                    )
                for c in range(NC):
                    nc.sync.dma_start(
                        xtb[:, c * T : (c + 1) * T], xT[c * 128 : (c + 1) * 128, :]
                    )

                # QT2/KT2: [128 = head-pair d dims, T]; 4 accumulators in flight
                for dst, wtile in ((qt, wqt), (kt_, wkt)):
                    for grp in range(2):
                        outs = [(grp, tb) for tb in range(NQB)]
                        pss = [
                            PS1.tile([128, 512], F32, tag="ps1", name=f"p1_{id(wtile)}_{grp}_{tb}")
                            for tb in range(NQB)
                        ]
                        for c in range(NC):
                            for o, (p, tb) in enumerate(outs):
                                nc.tensor.matmul(
                                    pss[o][:],
                                    wtile[:, c * 256 + p * 128 : c * 256 + p * 128 + 128],
                                    xtb[:, c * T + tb * 512 : c * T + tb * 512 + 512],
                                    start=(c == 0),
                                    stop=(c == NC - 1),
                                )
                        for o, (p, tb) in enumerate(outs):
                            nc.vector.tensor_copy(
                                dst[p][:, tb * 512 : (tb + 1) * 512], pss[o][:]
                            )
                # V: natural layout [t, d], 4 heads side by side (65-strided groups)
                for tg in range(NT // 2):
                    pss = [
                        PSV.tile([128, 256], F32, tag="psv", name=f"pv_{tg}_{o}")
                        for o in range(2)
                    ]
                    for c in range(NC):
                        for o in range(2):
                            tt = 2 * tg + o
                            nc.tensor.matmul(
                                pss[o][:],
                                xtb[:, c * T + tt * 128 : c * T + tt * 128 + 128],
                                wvt[:, c * 256 : (c + 1) * 256],
                                start=(c == 0),
                                stop=(c == NC - 1),
                            )
                    for o in range(2):
                        tt = 2 * tg + o
                        dst = v4[:, tt * 260 : (tt + 1) * 260].rearrange(
                            "p (l j) -> p l j", j=65
                        )[:, :, 0:64]
                        nc.scalar.copy(dst, pss[o][:].rearrange("p (l j) -> p l j", j=64))

            # ---------------- phase 2: attention (+ fused output proj) ----------------
            # Software-pipelined: iteration N's attn@V + normalize are emitted
            # after iteration N+1's scores/exp so the PE never waits on ACT.
            with (
                tc.tile_pool(name="ph2", bufs=3) as E2,
                tc.tile_pool(name="stg", bufs=1) as STG,
                tc.tile_pool(name="nrm", bufs=2) as NP,
                tc.tile_pool(name="ph3", bufs=3) as S3,
                tc.tile_pool(name="ps2", bufs=3, space="PSUM") as PS2,
                tc.tile_pool(name="pso", bufs=1, space="PSUM") as PSO,
            ):

                def emit_scores(qb, p):
                    e01 = [
                        E2.tile([128, NT * 512], MMDT, tag=f"e{l}", name=f"e{l}_{p}_{qb}")
                        for l in range(2)
                    ]
                    qs = qt[p][:, qb * 512 : (qb + 1) * 512]
                    for ktp in range(8):
                        sAB = [
                            PS2.tile([128, 1024], F32, tag="s", name=f"s{l}_{p}_{qb}_{ktp}")
                            for l in range(2)
                        ]
                        for half in range(2):
                            kti = 2 * ktp + half
                            for l in range(2):
                                lo, hi = 64 * l, 64 * l + 64
                                nc.tensor.matmul(
                                    sAB[l][:, half * 512 : half * 512 + 512],
                                    kt_[p][lo:hi, kti * 128 : kti * 128 + 128],
                                    qs[lo:hi, :],
                                    start=True,
                                    stop=True,
                                )
                        for l in range(2):
                            nc.scalar.activation(
                                e01[l][:, ktp * 1024 : ktp * 1024 + 1024],
                                sAB[l][:],
                                EXP,
                                scale=SCALE,
                            )
                    return e01

                def emit_tail(qb, p, e01):
                    po = [
                        PSO.tile([65, 512], F32, tag=f"po{l}", name=f"po{l}_{p}_{qb}")
                        for l in range(2)
                    ]
                    for kti in range(NT):
                        for l in range(2):
                            g0 = kti * 260 + (2 * p + l) * 65
                            nc.tensor.matmul(
                                po[l][:],
                                v4[:, g0 : g0 + 65],
                                e01[l][:, kti * 512 : kti * 512 + 512],
                                start=(kti == 0),
                                stop=(kti == NT - 1),
                                skip_group_check=True,
                            )
                    for l in range(2):
                        e = e01[l]
                        srow = NP.tile([1, 512], MMDT, tag="srow")
                        nc.vector.tensor_copy(srow[:], po[l][64:65, :])
                        rbps = PS2.tile([128, 512], F32, tag="s", name=f"rb_{p}_{qb}_{l}")
                        nc.tensor.matmul(rbps[:], onesrt[:], srow[:], start=True, stop=True)
                        rb = NP.tile([128, 512], F32, tag="rb")
                        nc.vector.reciprocal(rb[:], rbps[:])
                        rb2 = rb[:].unsqueeze(1).to_broadcast([128, 2, 512])
                        if l == 0:
                            nc.vector.tensor_tensor(
                                ot2[p][0:64, qb * 512 : qb * 512 + 512],
                                po[l][0:64, :],
                                rb[0:64, :],
                                op=mybir.AluOpType.mult,
                            )
                        else:
                            tmp = NP.tile([64, 512], MMDT, tag="tmp")
                            nc.vector.tensor_tensor(
                                tmp[:],
                                po[l][0:64, :],
                                rb[0:64, :],
                                op=mybir.AluOpType.mult,
                            )
                            nc.sync.dma_start(
                                ot2[p][64:128, qb * 512 : qb * 512 + 512], tmp[:]
                            )
                        lg = 2 * p + l
                        for hf in range(2):
                            stage = STG.tile(
                                [128, 8 * 512], F32, tag=f"st{l}",
                                name=f"st{l}{hf}_{p}_{qb}",
                            )
                            for kp in range(4):
                                eng = nc.gpsimd if kp >= 2 else nc.vector
                                eng.tensor_tensor(
                                    stage[:, kp * 1024 : kp * 1024 + 1024].rearrange(
                                        "p (a j) -> p a j", j=512
                                    ),
                                    e[:, hf * 4096 + kp * 1024 : hf * 4096 + kp * 1024 + 1024].rearrange(
                                        "p (a j) -> p a j", j=512
                                    ),
                                    rb2,
                                    op=mybir.AluOpType.mult,
                                )
                            nc.sync.dma_start(
                                attnT[lg].rearrange("(kt p) q -> p kt q", p=128)[
                                    :, hf * 8 : hf * 8 + 8, qb * 512 : qb * 512 + 512
                                ],
                                stage[:].rearrange("p (kt j) -> p kt j", j=512),
                            )
                    if p == 1:
                        for tt in range(4 * qb, 4 * qb + 4):
                            for db in range(2):
                                ps = PS2.tile([128, 512], F32, tag="s", name=f"p3_{tt}_{db}")
                                for pp in range(2):
                                    nc.tensor.matmul(
                                        ps[:],
                                        ot2[pp][:, tt * 128 : tt * 128 + 128],
                                        wot[:, pp * D + db * 512 : pp * D + db * 512 + 512],
                                        start=(pp == 0),
                                        stop=(pp == 1),
                                    )
                                st = S3.tile([128, 512], F32, tag="st")
                                nc.vector.tensor_copy(st[:], ps[:])
                                nc.sync.dma_start(
                                    outp[tt * 128 : tt * 128 + 128, db * 512 : db * 512 + 512],
                                    st[:],
                                )

                pending = None
                for qb in range(NQB):
                    for p in range(2):
                        e01 = emit_scores(qb, p)
                        if pending is not None:
                            emit_tail(*pending)
                        pending = (qb, p, e01)
                emit_tail(*pending)

    nc.compile()
    return nc


def kernel(x, W_q, W_k, W_v, W_o):
    global LAST_RESULT
    x = np.asarray(x, dtype=np.float32)
    W_q = np.asarray(W_q, dtype=np.float32)
    W_k = np.asarray(W_k, dtype=np.float32)
    W_v = np.asarray(W_v, dtype=np.float32)
    W_o = np.asarray(W_o, dtype=np.float32)

    nc = _build()

    ones = np.ones((128, 64), dtype=NPDT)
    onesr = np.ones((1, 128), dtype=NPDT)
    in_maps = []
    for c in range(8):
        b = c // 4
        hs = 4 * (c % 4)  # first global head of this core
        r0 = 64 * hs
        in_maps.append(
            {
                "xT": np.ascontiguousarray(x[b].T).astype(NPDT),
                "wq": np.ascontiguousarray(W_q[r0 : r0 + 256, :].T).astype(NPDT),
                "wk": np.ascontiguousarray(W_k[r0 : r0 + 256, :].T).astype(NPDT),
                "wv": np.ascontiguousarray(W_v[r0 : r0 + 256, :].T).astype(NPDT),
                "wo": np.ascontiguousarray(W_o[:, r0 : r0 + 256].T).astype(NPDT),
                "ones": ones,
                "onesr": onesr,
            }
        )

    res = run_bass_kernel_spmd(nc, in_maps, core_ids=list(range(8)))
    LAST_RESULT = res

    B, H = 2, 16
    out = np.zeros((B, T, D), dtype=np.float32)
    attn = np.empty((B, H, T, T), dtype=np.float32)
    for c in range(8):
        b = c // 4
        hs = 4 * (c % 4)
        r = res.results[c]
        out[b] += r["outp"]
        a = r["attnT"]  # [4, k, q]
        for l in range(HL):
            attn[b, hs + l] = a[l].T
    return out, attn
